# revision 14
# baseline (speedup 1.0000x reference)
"""NonLocal block (no-softmax attention) Trainium2 kernel.

Math: out = BN(W_rec @ ((theta^T phi / n) @ g)^T) + x, with theta/phi/g 1x1 convs.
Since there is no softmax, (theta^T phi) g reassociates to theta^T (phi g^T):
the n x n attention matrix collapses to a 128x128 Gram matrix K = phi @ g^T / n.

Per-batch, channel-major [C, n] layout:
  phi_sp/g_sp = X^T @ [Wphi^T/n | Wg^T] + biases    (spatial-major, [n, 256])
  K = phi_sp^T @ g_sp                               ([128, 128], contraction over n)
  theta = Wtheta @ X + btheta                       ([128, n])
  O = K^T @ theta                                   ([128, n])
  y = Wrec' @ O + brec' + X                         (BN folded into Wrec/brec)

Sharding: 8 cores = 4 batches x 2 spatial halves. Each core computes K for its
full batch (duplicated within the pair; avoids collectives) but theta/O/y only
for its half of the 3136 spatial positions. Inputs are host-permuted so each
core's half is the leading 1568 columns.

Matmuls run in float32r (TF32-like fast fp32 PE mode, 1 cyc/row at free-dim
>= 256 vs 4 for fp32). fp32r operands must come from fp32r-typed producers;
DMA'd tensors are host-rounded (RNE) onto the fp32r grid, on-device
PSUM->SBUF copies cast to fp32r.

Self-loading (4-byte-dtype) matmuls can carry at most ONE sync wait in the
ISA, so the program is structured so every matmul needs at most one fresh
semaphore: two dummy matmuls absorb the x-tile DMA waits onto the PE clock,
weights/biases bounce through DVE copies, per-chunk Gram matmuls follow each
PSUM->SBUF copy, and theta/rec biases ride on DVE copy ops instead of rank-1
matmuls.
"""

import numpy as np

BN_EPS = 1e-5
B, C, CI = 4, 256, 128
H = W = 56
N = H * W            # 3136 spatial positions
NH = N // 2          # 1568 per core
NT = 392             # stage-B free-dim tile (4 tiles of 392 = 1568)
NB_TILES = NH // NT
CHUNK = 128
NCHUNKS = (N + CHUNK - 1) // CHUNK   # 25 (24 full + one of 64)

USE_F32R = True      # run matmuls in float32r (fast fp32 mode on the PE)

_NC_CACHE = {}


def _tf32_round(a):
    """Round-to-nearest-even onto the 10-bit-mantissa fp32r grid."""
    bits = np.ascontiguousarray(a, np.float32).view(np.uint32).copy()
    lsb = (bits >> 13) & 1
    bits = (bits + 0x0FFF + lsb) & np.uint32(0xFFFFE000)
    return bits.view(np.float32)


def _build_nc(use_f32r):
    import concourse.mybir as mybir
    import concourse.tile as tile
    from concourse import bacc

    f32 = mybir.dt.float32
    mdt = mybir.dt.float32r if use_f32r else f32
    ADD = mybir.AluOpType.add

    nc = bacc.Bacc("TRN2", target_bir_lowering=False, debug=False)
    xp = nc.dram_tensor("xp", [C, N], mdt, kind="ExternalInput")
    w_pg = nc.dram_tensor("w_pg", [C, 2 * CI], mdt, kind="ExternalInput")
    w_th = nc.dram_tensor("w_th", [C, CI], mdt, kind="ExternalInput")
    w_rc = nc.dram_tensor("w_rc", [CI, C], mdt, kind="ExternalInput")
    b_pg = nc.dram_tensor("b_pg", [1, 2 * CI], mdt, kind="ExternalInput")
    b_thc = nc.dram_tensor("b_thc", [CI, 1], f32, kind="ExternalInput")
    b_rc2 = nc.dram_tensor("b_rc2", [128, 2], f32, kind="ExternalInput")
    y = nc.dram_tensor("y", [C, NH], f32, kind="ExternalOutput")

    with tile.TileContext(nc) as tc:
        with (
            tc.tile_pool(name="const", bufs=1) as constp,
            tc.tile_pool(name="xpool", bufs=1) as xpool,
            tc.tile_pool(name="pgpool", bufs=1) as pgpool,
            tc.tile_pool(name="work", bufs=2) as work,
            tc.tile_pool(name="psD", bufs=1, space="PSUM") as psD,
        ):
            # ---- DMA loads (weights first; x streamed in 392-col pieces so
            # stage-A matmuls can start after the first piece) ----
            w_pg_raw = []
            for i in range(2):
                wt = constp.tile([128, 2 * CI], mdt, name=f"w_pg_raw{i}")
                nc.sync.dma_start(wt[:], w_pg[i * 128:(i + 1) * 128, :])
                w_pg_raw.append(wt)
            w_th_raw = []
            for i in range(2):
                wt = constp.tile([128, CI], mdt, name=f"w_th_raw{i}")
                nc.sync.dma_start(wt[:], w_th[i * 128:(i + 1) * 128, :])
                w_th_raw.append(wt)
            w_rc_raw = constp.tile([CI, C], mdt)
            nc.sync.dma_start(w_rc_raw[:], w_rc[:])
            b_pg_raw = constp.tile([1, 2 * CI], mdt)
            nc.sync.dma_start(b_pg_raw[:], b_pg[:])
            b_thc_sb = constp.tile([CI, 1], f32)
            nc.sync.dma_start(b_thc_sb[:], b_thc[:])
            b_rc2_sb = constp.tile([128, 2], f32)
            nc.sync.dma_start(b_rc2_sb[:], b_rc2[:])

            x_sb = []
            for i in range(2):
                xt = xpool.tile([128, N], mdt, name=f"x_sb{i}")
                x_sb.append(xt)
            for p in range(N // NT):
                for i in range(2):
                    ps_ = slice(p * NT, (p + 1) * NT)
                    nc.sync.dma_start(x_sb[i][:, ps_], xp[i * 128:(i + 1) * 128, ps_])

            dum_ps = psD.tile([1, 1], f32)

            def dummy_mm(src):
                # N=1 fp32r matmuls violate ISA restrictions; run the
                # wait-absorbing dummies as plain fp32
                col = src[:, 0:1].bitcast(f32) if use_f32r else src[:, 0:1]
                nc.tensor.matmul(dum_ps[:], col, col, start=True, stop=True)

            # ---- DVE bounces: single PE-visible producer for weights ----
            w_pg_sb, w_th_sb = [], []
            for i in range(2):
                wt = constp.tile([128, 2 * CI], mdt, name=f"w_pg_sb{i}")
                nc.vector.tensor_copy(wt[:], w_pg_raw[i][:])
                w_pg_sb.append(wt)
            for i in range(2):
                wt = constp.tile([128, CI], mdt, name=f"w_th_sb{i}")
                nc.vector.tensor_copy(wt[:], w_th_raw[i][:])
                w_th_sb.append(wt)
            w_rc_sb = constp.tile([CI, C], mdt)
            nc.vector.tensor_copy(w_rc_sb[:], w_rc_raw[:])
            b_pg_sb = constp.tile([1, 2 * CI], mdt)
            nc.vector.tensor_copy(b_pg_sb[:], b_pg_raw[:])
            ones_f = constp.tile([1, CHUNK], f32)
            nc.vector.memset(ones_f[:], 1.0)
            ones = constp.tile([1, CHUNK], mdt)
            nc.vector.tensor_copy(ones[:], ones_f[:])

            # bridge: absorb the DVE bounce-chain wait onto the PE clock, so
            # each stage-A matmul needs at most the one fresh x-piece DMA wait
            dummy_mm(ones)

            # spatial-major [phi | g], all chunks kept in SBUF
            pg_all = pgpool.tile([128, NCHUNKS * 2 * CI], mdt)
            k_sb = constp.tile([CI, CI], mdt)

            # ---- stage A: phi/g spatial-major, Gram interleaved ----
            with tc.tile_pool(name="psA", bufs=4, space="PSUM") as psA:
                gram_ps = psA.tile([128, 2 * CI], f32, tag="gram", bufs=1)

                def gram_mm(kc):
                    s = min(CHUNK, N - kc * CHUNK)
                    base = kc * 2 * CI
                    nc.tensor.matmul(
                        gram_ps[:, :],
                        pg_all[:s, base:base + CI],
                        pg_all[:s, base:base + 2 * CI],
                        start=(kc == 0), stop=(kc == NCHUNKS - 1),
                    )

                for kc in range(NCHUNKS):
                    s = min(CHUNK, N - kc * CHUNK)
                    cs = slice(kc * CHUNK, kc * CHUNK + s)
                    ps = psA.tile([128, 2 * CI], f32, tag="psA")
                    nc.tensor.matmul(ps[:s, :], x_sb[0][:, cs], w_pg_sb[0][:],
                                     start=True, stop=False)
                    nc.tensor.matmul(ps[:s, :], x_sb[1][:, cs], w_pg_sb[1][:],
                                     start=False, stop=False)
                    nc.tensor.matmul(ps[:s, :], ones[:, :s], b_pg_sb[:],
                                     start=False, stop=True)
                    dst = pg_all[:s, kc * 2 * CI:(kc + 1) * 2 * CI]
                    if kc % 2 == 0:
                        nc.vector.tensor_copy(dst, ps[:s, :])
                    else:
                        nc.scalar.copy(dst, ps[:s, :])
                    if kc >= 1:
                        gram_mm(kc - 1)
                gram_mm(NCHUNKS - 1)
                nc.vector.tensor_copy(k_sb[:], gram_ps[:, CI:2 * CI])
                # bridge: absorb the k_sb-copy DVE wait onto the PE clock so
                # stage-B matmuls (which also carry a PSUM bank-reuse wait)
                # stay within the single-wait ISA limit
                dummy_mm(k_sb)

            # ---- stage B: theta -> O -> y on this core's half ----
            with tc.tile_pool(name="psB", bufs=2, space="PSUM") as psB:
                for t in range(NB_TILES):
                    ts_ = slice(t * NT, (t + 1) * NT)
                    th_ps = psB.tile([CI, NT], f32, tag="th")
                    nc.tensor.matmul(th_ps[:], w_th_sb[0][:], x_sb[0][:, ts_],
                                     start=True, stop=False)
                    nc.tensor.matmul(th_ps[:], w_th_sb[1][:], x_sb[1][:, ts_],
                                     start=False, stop=True)
                    th_sb = work.tile([CI, NT], mdt, tag="th_sb")
                    # theta bias is per-partition: fold into the copy
                    nc.vector.tensor_scalar(th_sb[:], th_ps[:], b_thc_sb[:],
                                            None, ADD)

                    o_ps = psB.tile([CI, NT], f32, tag="o")
                    nc.tensor.matmul(o_ps[:], k_sb[:], th_sb[:],
                                     start=True, stop=True)
                    o_sb = work.tile([CI, NT], mdt, tag="o_sb")
                    nc.vector.tensor_copy(o_sb[:], o_ps[:])

                    for oc in range(2):
                        y_ps = psB.tile([128, NT], f32, tag="y")
                        nc.tensor.matmul(y_ps[:], w_rc_sb[:, oc * 128:(oc + 1) * 128],
                                         o_sb[:], start=True, stop=True)
                        y_sb = work.tile([128, NT], f32, tag="y_sb")
                        xres = x_sb[oc][:, ts_]
                        if use_f32r:
                            xres = xres.bitcast(f32)
                        # y = (y_ps + b_rc[oc]) + x  in one DVE op
                        nc.vector.scalar_tensor_tensor(
                            y_sb[:], y_ps[:], b_rc2_sb[:, oc:oc + 1], xres,
                            ADD, ADD)
                        nc.sync.dma_start(y[oc * 128:(oc + 1) * 128, ts_], y_sb[:])
    nc.finalize()
    return nc


def _get_nc():
    key = USE_F32R
    if key not in _NC_CACHE:
        _NC_CACHE[key] = _build_nc(USE_F32R)
    return _NC_CACHE[key]


def kernel(x, w_theta, b_theta, w_phi, b_phi, w_g, b_g,
           w_rec, b_rec, bn_gamma, bn_beta, bn_mean, bn_var):
    from concourse.bass_utils import run_bass_kernel_spmd

    x = np.asarray(x, np.float32)
    rnd = _tf32_round if USE_F32R else (lambda a: np.ascontiguousarray(a, np.float32))
    n = N
    inv = np.asarray(bn_gamma, np.float32) / np.sqrt(np.asarray(bn_var, np.float32) + BN_EPS)
    w_rec_f = inv[:, None] * np.asarray(w_rec, np.float32)
    b_rec_f = np.asarray(b_rec, np.float32) * inv + np.asarray(bn_beta, np.float32) \
        - np.asarray(bn_mean, np.float32) * inv

    cst = {
        "w_pg": rnd(np.concatenate([np.asarray(w_phi, np.float32).T / n,
                                    np.asarray(w_g, np.float32).T], axis=1)),
        "w_th": rnd(np.asarray(w_theta, np.float32).T),
        "w_rc": rnd(w_rec_f.T),
        "b_pg": rnd(np.concatenate([np.asarray(b_phi, np.float32) / n,
                                    np.asarray(b_g, np.float32)])[None, :]),
        "b_thc": np.ascontiguousarray(np.asarray(b_theta, np.float32)[:, None]),
        "b_rc2": np.ascontiguousarray(b_rec_f.reshape(2, 128).T),
    }

    xf = x.reshape(B, C, n)
    in_maps = []
    for core in range(8):
        b_i, h_i = divmod(core, 2)
        if h_i == 0:
            xpm = xf[b_i]
        else:
            xpm = np.concatenate([xf[b_i][:, NH:], xf[b_i][:, :NH]], axis=1)
        in_maps.append({"xp": rnd(xpm), **cst})

    res = run_bass_kernel_spmd(_get_nc(), in_maps, core_ids=list(range(8)))

    out = np.empty((B, C, n), np.float32)
    for core in range(8):
        b_i, h_i = divmod(core, 2)
        out[b_i][:, h_i * NH:(h_i + 1) * NH] = res.results[core]["y"]
    return out.reshape(B, C, H, W)


# revision 17
# speedup vs baseline: 1.7326x; 1.7326x over previous
"""NonLocal block (no-softmax attention) Trainium2 kernel.

Math: out = BN(W_rec @ ((theta^T phi / n) @ g)^T) + x, with theta/phi/g 1x1 convs.
Since there is no softmax, (theta^T phi) g reassociates to theta^T (phi g^T):
the n x n attention matrix collapses to a 128x128 Gram matrix K = phi @ g^T / n.

Per-batch, channel-major [C, n] layout:
  phi_sp/g_sp = X^T @ [Wphi^T/n | Wg^T] + biases    (spatial-major, [n, 256])
  K^T = g_sp^T @ phi_sp                             ([128, 128], contraction over n)
  M2^T = K^T_lhsT-form @ Wrec'^T = (Wrec' K^T)^T    ([128, 256], folds O into y)
  theta = Wtheta @ X + btheta                       ([128, n])
  y = M2 @ theta + brec' + X                        (BN folded into Wrec/brec)

Sharding: 8 cores = 4 batches x 2 spatial halves. Each core computes K for its
full batch (duplicated within the pair; avoids collectives) but theta/y only
for its half of the 3136 spatial positions. Inputs are host-permuted so each
core's half is the leading 1568 columns.

Matmul inputs are fp16: same 11-bit mantissa as the PE's fast-fp32 (fp32r/TF32)
mode, but at 1 cycle/row (vs 2), with HAM warm-up, FWL weight loads, and half
the DMA bytes. Accumulation stays fp32 in PSUM; residual add + output are fp32.
End-to-end scaled absmax error vs the fp32 reference: ~7e-4.

Structure notes:
- x is streamed in 392-col DMA pieces so stage-A matmuls start early.
- phi/g biases ride on the PSUM->SBUF copy (DVE tensor_tensor add with a
  broadcast bias tile) instead of rank-1 matmuls.
- theta bias rides on the Scalar-engine PSUM->SBUF copy (per-partition bias).
- rec bias + residual ride on one DVE scalar_tensor_tensor per output tile.
- Gram matmuls are interleaved into the stage-A chunk loop (accumulating
  PSUM group with other matmuls in between is fine on HW).
"""

import numpy as np

BN_EPS = 1e-5
B, C, CI = 4, 256, 128
H = W = 56
N = H * W            # 3136 spatial positions
NH = N // 2          # 1568 per core
NT = 392             # stage-B free-dim tile (4 tiles of 392 = 1568)
NB_TILES = NH // NT
CHUNK = 128
NCHUNKS = (N + CHUNK - 1) // CHUNK   # 25 (24 full + one of 64)
NPAIRS = (NCHUNKS + 1) // 2          # 13 (12 pairs + 1 single)

MODE = "f16"         # "f16" | "bf16" | "f32"

_NC_CACHE = {}


def _host_cast(mode):
    if mode == "f16":
        return lambda a: np.ascontiguousarray(np.asarray(a, np.float32).astype(np.float16))
    if mode == "bf16":
        import ml_dtypes
        return lambda a: np.ascontiguousarray(
            np.asarray(a, np.float32).astype(ml_dtypes.bfloat16))
    return lambda a: np.ascontiguousarray(a, np.float32)


def _build_nc(mode):
    import concourse.mybir as mybir
    import concourse.tile as tile
    from concourse import bacc

    f32 = mybir.dt.float32
    mdt = {"f16": mybir.dt.float16, "bf16": mybir.dt.bfloat16, "f32": f32}[mode]
    ADD = mybir.AluOpType.add
    IDENT = mybir.ActivationFunctionType.Identity

    nc = bacc.Bacc("TRN2", target_bir_lowering=False, debug=False)
    xp = nc.dram_tensor("xp", [C, N], mdt, kind="ExternalInput")
    w_pg = nc.dram_tensor("w_pg", [C, 2 * CI], mdt, kind="ExternalInput")
    w_th = nc.dram_tensor("w_th", [C, CI], mdt, kind="ExternalInput")
    w_rc = nc.dram_tensor("w_rc", [CI, C], mdt, kind="ExternalInput")
    b_pgt = nc.dram_tensor("b_pgt", [128, 4 * CI], mdt, kind="ExternalInput")
    b_thc = nc.dram_tensor("b_thc", [CI, 1], f32, kind="ExternalInput")
    b_rc2 = nc.dram_tensor("b_rc2", [128, 2], f32, kind="ExternalInput")
    y = nc.dram_tensor("y", [C, NH], f32, kind="ExternalOutput")

    with tile.TileContext(nc) as tc:
        with (
            tc.tile_pool(name="const", bufs=1) as constp,
            tc.tile_pool(name="xpool", bufs=1) as xpool,
            tc.tile_pool(name="pgpool", bufs=1) as pgpool,
            tc.tile_pool(name="work", bufs=3) as work,
        ):
            # ---- DMA loads (weights first; x streamed in 392-col pieces so
            # stage-A matmuls can start after the first piece) ----
            w_pg_sb = []
            for i in range(2):
                wt = constp.tile([128, 2 * CI], mdt, name=f"w_pg_sb{i}")
                nc.sync.dma_start(wt[:], w_pg[i * 128:(i + 1) * 128, :])
                w_pg_sb.append(wt)
            w_th_sb = []
            for i in range(2):
                wt = constp.tile([128, CI], mdt, name=f"w_th_sb{i}")
                nc.sync.dma_start(wt[:], w_th[i * 128:(i + 1) * 128, :])
                w_th_sb.append(wt)
            w_rc_sb = constp.tile([CI, C], mdt)
            nc.sync.dma_start(w_rc_sb[:], w_rc[:])
            b_pgt_sb = constp.tile([128, 4 * CI], mdt)
            nc.sync.dma_start(b_pgt_sb[:], b_pgt[:])
            b_thc_sb = constp.tile([CI, 1], f32)
            nc.sync.dma_start(b_thc_sb[:], b_thc[:])
            b_rc2_sb = constp.tile([128, 2], f32)
            nc.sync.dma_start(b_rc2_sb[:], b_rc2[:])

            x_sb = []
            for i in range(2):
                xt = xpool.tile([128, N], mdt, name=f"x_sb{i}")
                x_sb.append(xt)
            for p in range(N // NT):
                for i in range(2):
                    ps_ = slice(p * NT, (p + 1) * NT)
                    nc.sync.dma_start(x_sb[i][:, ps_], xp[i * 128:(i + 1) * 128, ps_])

            # spatial-major [phi | g], all chunks kept in SBUF
            pg_all = pgpool.tile([128, NCHUNKS * 2 * CI], mdt)
            kt_sb = constp.tile([CI, CI], mdt)       # K^T = g_sp^T phi_sp
            m2t_sb = constp.tile([CI, C], mdt)       # (Wrec' K^T)^T

            # ---- stage A: phi/g spatial-major, Gram interleaved ----
            with tc.tile_pool(name="psA", bufs=3, space="PSUM") as psA:
                gram_ps = psA.tile([128, 2 * CI], f32, tag="gram", bufs=1)

                def gram_mm(kc):
                    s = min(CHUNK, N - kc * CHUNK)
                    base = kc * 2 * CI
                    # lhsT = g part -> out = [G Phi^T | G G^T]; K^T is cols 0:CI
                    nc.tensor.matmul(
                        gram_ps[:, :],
                        pg_all[:s, base + CI:base + 2 * CI],
                        pg_all[:s, base:base + 2 * CI],
                        start=(kc == 0), stop=(kc == NCHUNKS - 1),
                    )

                for pr in range(NPAIRS):
                    c0 = 2 * pr
                    chunks = [c for c in (c0, c0 + 1) if c < NCHUNKS]
                    width = 256 * len(chunks)
                    ps = psA.tile([128, 2 * 2 * CI], f32, tag="psA")
                    smin = 128
                    for ci_, kc in enumerate(chunks):
                        s = min(CHUNK, N - kc * CHUNK)
                        smin = min(smin, s)
                        cs = slice(kc * CHUNK, kc * CHUNK + s)
                        off = ci_ * 2 * CI
                        nc.tensor.matmul(ps[:s, off:off + 2 * CI],
                                         x_sb[0][:, cs], w_pg_sb[0][:],
                                         start=True, stop=False)
                        nc.tensor.matmul(ps[:s, off:off + 2 * CI],
                                         x_sb[1][:, cs], w_pg_sb[1][:],
                                         start=False, stop=True)
                    # copy with phi/g bias add (bias along free dim, so a
                    # broadcast bias tile + tensor_tensor on DVE)
                    s = 128 if len(chunks) == 2 else smin
                    nc.vector.tensor_tensor(
                        pg_all[:s, c0 * 2 * CI: c0 * 2 * CI + width],
                        ps[:s, :width], b_pgt_sb[:s, :width], ADD)
                    for kc in chunks:
                        gram_mm(kc)

                nc.vector.tensor_copy(kt_sb[:], gram_ps[:, 0:CI])
                m2_ps = psA.tile([CI, C], f32, tag="m2", bufs=1)
                nc.tensor.matmul(m2_ps[:], kt_sb[:], w_rc_sb[:],
                                 start=True, stop=True)
                nc.vector.tensor_copy(m2t_sb[:], m2_ps[:])

            # ---- stage B: theta -> y on this core's half ----
            with tc.tile_pool(name="psB", bufs=2, space="PSUM") as psB:
                for t in range(NB_TILES):
                    ts_ = slice(t * NT, (t + 1) * NT)
                    th_ps = psB.tile([CI, NT], f32, tag="th")
                    nc.tensor.matmul(th_ps[:], w_th_sb[0][:], x_sb[0][:, ts_],
                                     start=True, stop=False)
                    nc.tensor.matmul(th_ps[:], w_th_sb[1][:], x_sb[1][:, ts_],
                                     start=False, stop=True)
                    th_sb = work.tile([CI, NT], mdt, tag="th_sb")
                    # theta bias is per-partition: ride it on the ACT copy
                    nc.scalar.activation(th_sb[:], th_ps[:], IDENT,
                                         bias=b_thc_sb[:])

                    for oc in range(2):
                        y_ps = psB.tile([128, NT], f32, tag="y")
                        nc.tensor.matmul(y_ps[:], m2t_sb[:, oc * 128:(oc + 1) * 128],
                                         th_sb[:], start=True, stop=True)
                        y_sb = work.tile([128, NT], f32, tag="y_sb")
                        # y = (y_ps + b_rc[oc]) + x  in one DVE op
                        nc.vector.scalar_tensor_tensor(
                            y_sb[:], y_ps[:], b_rc2_sb[:, oc:oc + 1],
                            x_sb[oc][:, ts_], ADD, ADD)
                        nc.sync.dma_start(y[oc * 128:(oc + 1) * 128, ts_], y_sb[:])
    nc.finalize()
    return nc


def _get_nc():
    if MODE not in _NC_CACHE:
        _NC_CACHE[MODE] = _build_nc(MODE)
    return _NC_CACHE[MODE]


def kernel(x, w_theta, b_theta, w_phi, b_phi, w_g, b_g,
           w_rec, b_rec, bn_gamma, bn_beta, bn_mean, bn_var):
    from concourse.bass_utils import run_bass_kernel_spmd

    x = np.asarray(x, np.float32)
    cast = _host_cast(MODE)
    n = N
    inv = np.asarray(bn_gamma, np.float32) / np.sqrt(np.asarray(bn_var, np.float32) + BN_EPS)
    w_rec_f = inv[:, None] * np.asarray(w_rec, np.float32)
    b_rec_f = np.asarray(b_rec, np.float32) * inv + np.asarray(bn_beta, np.float32) \
        - np.asarray(bn_mean, np.float32) * inv

    b_pg_row = np.concatenate([np.asarray(b_phi, np.float32) / n,
                               np.asarray(b_g, np.float32)])          # [256]
    cst = {
        "w_pg": cast(np.concatenate([np.asarray(w_phi, np.float32).T / n,
                                     np.asarray(w_g, np.float32).T], axis=1)),
        "w_th": cast(np.asarray(w_theta, np.float32).T),
        "w_rc": cast(w_rec_f.T),
        "b_pgt": cast(np.tile(np.concatenate([b_pg_row, b_pg_row])[None, :],
                              (128, 1))),
        "b_thc": np.ascontiguousarray(np.asarray(b_theta, np.float32)[:, None]),
        "b_rc2": np.ascontiguousarray(b_rec_f.reshape(2, 128).T),
    }

    xf = x.reshape(B, C, n)
    in_maps = []
    for core in range(8):
        b_i, h_i = divmod(core, 2)
        if h_i == 0:
            xpm = xf[b_i]
        else:
            xpm = np.concatenate([xf[b_i][:, NH:], xf[b_i][:, :NH]], axis=1)
        in_maps.append({"xp": cast(xpm), **cst})

    res = run_bass_kernel_spmd(_get_nc(), in_maps, core_ids=list(range(8)))

    out = np.empty((B, C, n), np.float32)
    for core in range(8):
        b_i, h_i = divmod(core, 2)
        out[b_i][:, h_i * NH:(h_i + 1) * NH] = res.results[core]["y"]
    return out.reshape(B, C, H, W)


# revision 21
# speedup vs baseline: 1.9605x; 1.1315x over previous
"""NonLocal block (no-softmax attention) Trainium2 kernel.

Math: out = BN(W_rec @ ((theta^T phi / n) @ g)^T) + x, with theta/phi/g 1x1 convs.
Since there is no softmax, (theta^T phi) g reassociates to theta^T (phi g^T):
the n x n attention matrix collapses to a 128x128 Gram matrix K = phi @ g^T / n.

Per-batch, channel-major [C, n] layout:
  phi_sp/g_sp = X^T @ [Wphi^T/n | Wg^T] + biases    (spatial-major, [n, 256])
  K^T = g_sp^T @ phi_sp                             ([128, 128], contraction over n)
  M2^T = K^T_lhsT-form @ Wrec'^T = (Wrec' K^T)^T    ([128, 256], folds O into y)
  theta = Wtheta @ X + btheta                       ([128, n])
  y = M2 @ theta + brec' + X                        (BN folded into Wrec/brec)

Sharding: 8 cores = 4 batches x 2 spatial halves. Each core computes K for its
full batch (duplicated within the pair; avoids collectives) but theta/y only
for its half of the 3136 spatial positions. Inputs are host-permuted so each
core's half is the leading 1568 columns.

Matmul inputs are fp16: same 11-bit mantissa as the PE's fast-fp32 (fp32r/TF32)
mode, but at 1 cycle/row (vs 2), with HAM warm-up, FWL weight loads, and half
the DMA bytes. Accumulation stays fp32 in PSUM; residual add + output are fp32.
End-to-end scaled absmax error vs the fp32 reference: ~7e-4.

Structure notes:
- x is streamed in 392-col DMA pieces so stage-A matmuls start early.
- phi/g biases ride on the PSUM->SBUF copy (DVE tensor_tensor add with a
  broadcast bias tile) instead of rank-1 matmuls.
- theta bias rides on the Scalar-engine PSUM->SBUF copy (per-partition bias).
- rec bias + residual ride on one DVE scalar_tensor_tensor per output tile.
- Gram matmuls are interleaved into the stage-A chunk loop (accumulating
  PSUM group with other matmuls in between is fine on HW).
"""

import numpy as np

BN_EPS = 1e-5
B, C, CI = 4, 256, 128
H = W = 56
N = H * W            # 3136 spatial positions
NH = N // 2          # 1568 per core
NT = 392             # stage-B free-dim tile (4 tiles of 392 = 1568)
NB_TILES = NH // NT
CHUNK = 128
NCHUNKS = (N + CHUNK - 1) // CHUNK   # 25 (24 full + one of 64)
NPAIRS = (NCHUNKS + 1) // 2          # 13 (12 pairs + 1 single)

MODE = "f16"         # "f16" | "bf16" | "f32"

_NC_CACHE = {}


def _host_cast(mode):
    if mode == "f16":
        return lambda a: np.ascontiguousarray(np.asarray(a, np.float32).astype(np.float16))
    if mode == "bf16":
        import ml_dtypes
        return lambda a: np.ascontiguousarray(
            np.asarray(a, np.float32).astype(ml_dtypes.bfloat16))
    return lambda a: np.ascontiguousarray(a, np.float32)


def _build_nc(mode):
    import concourse.mybir as mybir
    import concourse.tile as tile
    from concourse import bacc

    f32 = mybir.dt.float32
    mdt = {"f16": mybir.dt.float16, "bf16": mybir.dt.bfloat16, "f32": f32}[mode]
    ADD = mybir.AluOpType.add
    IDENT = mybir.ActivationFunctionType.Identity

    nc = bacc.Bacc("TRN2", target_bir_lowering=False, debug=False)
    xp = nc.dram_tensor("xp", [C, N], mdt, kind="ExternalInput")
    # all f16 weights/biases coalesced into one [128, 1536] DMA:
    # cols 0:256 w_pg c-chunk0 | 256:512 w_pg c-chunk1 | 512:640 w_th c0 |
    # 640:768 w_th c1 | 768:1024 w_rc | 1024:1536 phi/g bias tile
    wk = nc.dram_tensor("wk", [128, 1536], mdt, kind="ExternalInput")
    bk = nc.dram_tensor("bk", [128, 3], f32, kind="ExternalInput")
    y = nc.dram_tensor("y", [C, NH], f32, kind="ExternalOutput")

    with tile.TileContext(nc) as tc:
        with (
            tc.tile_pool(name="const", bufs=1) as constp,
            tc.tile_pool(name="xpool", bufs=1) as xpool,
            tc.tile_pool(name="pgpool", bufs=1) as pgpool,
            tc.tile_pool(name="work", bufs=3) as work,
        ):
            # ---- DMA loads: 2 coalesced const DMAs + x in quarters so
            # stage-A matmuls start after the first quarter ----
            wk_sb = constp.tile([128, 1536], mdt)
            nc.sync.dma_start(wk_sb[:], wk[:])
            bk_sb = constp.tile([128, 3], f32)
            nc.sync.dma_start(bk_sb[:], bk[:])
            w_pg_sb = [wk_sb[:, 0:256], wk_sb[:, 256:512]]
            w_th_sb = [wk_sb[:, 512:640], wk_sb[:, 640:768]]
            w_rc_sb = wk_sb[:, 768:1024]
            b_pgt_sb = wk_sb[:, 1024:1536]
            b_thc_sb = bk_sb[:, 0:1]
            b_rc2_sb = bk_sb[:, 1:3]

            XQ = N // 4  # 784-col x quarters
            x_sb = []
            for i in range(2):
                xt = xpool.tile([128, N], mdt, name=f"x_sb{i}")
                x_sb.append(xt)
            for p in range(4):
                for i in range(2):
                    ps_ = slice(p * XQ, (p + 1) * XQ)
                    nc.sync.dma_start(x_sb[i][:, ps_], xp[i * 128:(i + 1) * 128, ps_])

            # spatial-major [phi | g], all chunks kept in SBUF
            pg_all = pgpool.tile([128, NCHUNKS * 2 * CI], mdt)
            kt_sb = constp.tile([CI, CI], mdt)       # K^T = g_sp^T phi_sp
            m2t_sb = constp.tile([CI, C], mdt)       # (Wrec' K^T)^T

            # ---- single PSUM pool: psA(2) gram(1) m2(1) th(2) y(2) = 8 banks
            with tc.tile_pool(name="psum", bufs=1, space="PSUM") as psp:
                gram_ps = psp.tile([128, 2 * CI], f32, tag="gram", bufs=1)

                def gram_mm(kc):
                    s = min(CHUNK, N - kc * CHUNK)
                    base = kc * 2 * CI
                    # lhsT = g part -> out = [G Phi^T | G G^T]; K^T is cols 0:CI
                    nc.tensor.matmul(
                        gram_ps[:, :],
                        pg_all[:s, base + CI:base + 2 * CI],
                        pg_all[:s, base:base + 2 * CI],
                        start=(kc == 0), stop=(kc == NCHUNKS - 1),
                    )

                # ---- stage A: phi/g spatial-major, Gram interleaved ----
                for pr in range(NPAIRS):
                    c0 = 2 * pr
                    chunks = [c for c in (c0, c0 + 1) if c < NCHUNKS]
                    width = 256 * len(chunks)
                    ps = psp.tile([128, 2 * 2 * CI], f32, tag="psA", bufs=2)
                    smin = 128
                    for ci_, kc in enumerate(chunks):
                        s = min(CHUNK, N - kc * CHUNK)
                        smin = min(smin, s)
                        cs = slice(kc * CHUNK, kc * CHUNK + s)
                        off = ci_ * 2 * CI
                        nc.tensor.matmul(ps[:s, off:off + 2 * CI],
                                         x_sb[0][:, cs], w_pg_sb[0],
                                         start=True, stop=False)
                        nc.tensor.matmul(ps[:s, off:off + 2 * CI],
                                         x_sb[1][:, cs], w_pg_sb[1],
                                         start=False, stop=True)
                    # copy with phi/g bias add (bias along free dim, so a
                    # broadcast bias tile + tensor_tensor on DVE)
                    s = 128 if len(chunks) == 2 else smin
                    nc.vector.tensor_tensor(
                        pg_all[:s, c0 * 2 * CI: c0 * 2 * CI + width],
                        ps[:s, :width], b_pgt_sb[:s, :width], ADD)
                    for kc in chunks:
                        gram_mm(kc)

                # ---- theta for all tiles (independent of K: overlaps the
                # gram -> kt -> m2 chain) ----
                th_sbs = []
                for t in range(NB_TILES):
                    ts_ = slice(t * NT, (t + 1) * NT)
                    th_ps = psp.tile([CI, NT], f32, tag="th", bufs=2)
                    nc.tensor.matmul(th_ps[:], w_th_sb[0], x_sb[0][:, ts_],
                                     start=True, stop=False)
                    nc.tensor.matmul(th_ps[:], w_th_sb[1], x_sb[1][:, ts_],
                                     start=False, stop=True)
                    th_sb = work.tile([CI, NT], mdt, tag="th_sb", bufs=4)
                    # theta bias is per-partition: ride it on the ACT copy
                    nc.scalar.activation(th_sb[:], th_ps[:], IDENT,
                                         bias=b_thc_sb)
                    th_sbs.append(th_sb)

                nc.vector.tensor_copy(kt_sb[:], gram_ps[:, 0:CI])
                m2_ps = psp.tile([CI, C], f32, tag="m2", bufs=1)
                nc.tensor.matmul(m2_ps[:], kt_sb[:], w_rc_sb,
                                 start=True, stop=True)
                nc.vector.tensor_copy(m2t_sb[:], m2_ps[:])

                # ---- y = M2 @ theta + brec' + x ----
                y_sbs = [work.tile([128, NH], f32, name=f"y_sb{oc}", bufs=1)
                         for oc in range(2)]
                for t in range(NB_TILES):
                    ts_ = slice(t * NT, (t + 1) * NT)
                    for oc in range(2):
                        y_ps = psp.tile([128, NT], f32, tag="y", bufs=2)
                        nc.tensor.matmul(y_ps[:], m2t_sb[:, oc * 128:(oc + 1) * 128],
                                         th_sbs[t][:], start=True, stop=True)
                        # y = (y_ps + b_rc[oc]) + x  in one DVE op
                        nc.vector.scalar_tensor_tensor(
                            y_sbs[oc][:, ts_], y_ps[:], b_rc2_sb[:, oc:oc + 1],
                            x_sb[oc][:, ts_], ADD, ADD)
                for oc in range(2):
                    nc.sync.dma_start(y[oc * 128:(oc + 1) * 128, :], y_sbs[oc][:])
    nc.finalize()
    return nc


def _get_nc():
    if MODE not in _NC_CACHE:
        _NC_CACHE[MODE] = _build_nc(MODE)
    return _NC_CACHE[MODE]


def kernel(x, w_theta, b_theta, w_phi, b_phi, w_g, b_g,
           w_rec, b_rec, bn_gamma, bn_beta, bn_mean, bn_var):
    from concourse.bass_utils import run_bass_kernel_spmd

    x = np.asarray(x, np.float32)
    cast = _host_cast(MODE)
    n = N
    inv = np.asarray(bn_gamma, np.float32) / np.sqrt(np.asarray(bn_var, np.float32) + BN_EPS)
    w_rec_f = inv[:, None] * np.asarray(w_rec, np.float32)
    b_rec_f = np.asarray(b_rec, np.float32) * inv + np.asarray(bn_beta, np.float32) \
        - np.asarray(bn_mean, np.float32) * inv

    b_pg_row = np.concatenate([np.asarray(b_phi, np.float32) / n,
                               np.asarray(b_g, np.float32)])          # [256]
    w_pg_t = np.concatenate([np.asarray(w_phi, np.float32).T / n,
                             np.asarray(w_g, np.float32).T], axis=1)  # [256, 256]
    w_th_t = np.asarray(w_theta, np.float32).T                        # [256, 128]
    wk = np.concatenate([
        w_pg_t[:128], w_pg_t[128:],                                   # 0:256, 256:512
        w_th_t[:128], w_th_t[128:],                                   # 512:640, 640:768
        w_rec_f.T,                                                    # 768:1024
        np.tile(np.concatenate([b_pg_row, b_pg_row])[None, :], (128, 1)),  # 1024:1536
    ], axis=1)
    bk = np.concatenate([
        np.asarray(b_theta, np.float32)[:, None],
        b_rec_f.reshape(2, 128).T,
    ], axis=1)
    cst = {"wk": cast(wk), "bk": np.ascontiguousarray(bk)}

    xf = x.reshape(B, C, n)
    in_maps = []
    for core in range(8):
        b_i, h_i = divmod(core, 2)
        if h_i == 0:
            xpm = xf[b_i]
        else:
            xpm = np.concatenate([xf[b_i][:, NH:], xf[b_i][:, :NH]], axis=1)
        in_maps.append({"xp": cast(xpm), **cst})

    res = run_bass_kernel_spmd(_get_nc(), in_maps, core_ids=list(range(8)))

    out = np.empty((B, C, n), np.float32)
    for core in range(8):
        b_i, h_i = divmod(core, 2)
        out[b_i][:, h_i * NH:(h_i + 1) * NH] = res.results[core]["y"]
    return out.reshape(B, C, H, W)
